# revision 2
# baseline (speedup 1.0000x reference)
"""Grouped-experts SwiGLU MoE kernel for Trainium2 (8 NeuronCores).

Problem: T=8192 tokens (pre-sorted into contiguous per-expert blocks of
sizes num_tokens_per_expert), D=1024, H=2816, E=8 experts.
out[t] = (silu(x@w1^T) * (x@w3^T)) @ w2^T  with the owning expert's weights;
tokens past sum(counts) produce zeros.

Sharding: 8-way tensor-parallel split of the hidden dim H (padded
2816 -> 3072 = 24 tiles of 128; each core owns 3 h-tiles of every expert).
Every core processes ALL valid tokens of ALL experts for its h-slice and
emits partial outputs (contraction over h is split); the host sums the 8
partials.  This makes every core's instruction stream identical (true SPMD)
while doing only token-proportional work per expert -- perfectly
load-balanced regardless of how unbalanced the expert counts are.

Schedule notes (v2):
- GEMM2 is token-streaming (stationary w2 128x128 tiles, streaming h2), so
  its PE cost is 24*tokens cycles with no ceil(count/128) quantization, and
  it is emitted per ~512-token chunk right after that chunk's h2 is ready.
- Output stores go through the SWDGE (gpsimd/Pool) queue so they never
  head-of-line-block the next expert's loads on the SP HWDGE queue.
- Expert order interleaves large and small counts so a large expert's
  compute sits at the rep tail and hides the next rep's prefetch.
- GEMMs run in bf16 (PE 1 cycle/column) with fp32 PSUM accumulation.
"""

import sys

sys.path.insert(0, "/opt/trn_rl_repo")

import numpy as np
import ml_dtypes

T, D, E = 8192, 1024, 8
H = 2816
CAP = T // E
NCORES = 8
HT = 3  # h-tiles of 128 per core (24 total, 22 real + 2 zero pads)
HSLICE = HT * 128  # 384
BF16 = ml_dtypes.bfloat16

_COMPILE_CACHE = {}
LAST_RESULTS = None  # BassKernelResults of the most recent device run


def _derive_cfg(counts):
    """Static structure derived from the per-expert token counts.
    Tokens are packed exactly (no padding): expert e owns packed columns
    [offs[e], offs[e]+counts[e]).  GEMM1/3 and GEMM2 consume near-equal
    chunks of <=512 columns."""
    counts = [int(c) for c in counts]
    offs = [0]
    for c in counts:
        offs.append(offs[-1] + c)
    total_cols = offs[-1]
    chunks = {}   # e -> [(col0, width<=512)]
    for e in range(E):
        c = counts[e]
        chunks[e] = []
        if c == 0:
            continue
        n = -(-c // 512)
        base, rem = divmod(c, n)
        c0 = 0
        for i in range(n):
            w = base + (1 if i < rem else 0)
            chunks[e].append((offs[e] + c0, w))
            c0 += w
    # big, small, big, ... so a large expert sits at the rep tail
    desc = sorted(range(E), key=lambda e: -counts[e])
    order = []
    lo, hi = 0, E - 1
    while lo <= hi:
        order.append(desc[lo])
        if hi != lo:
            order.append(desc[hi])
        lo += 1
        hi -= 1
    order = [e for e in order if counts[e] > 0]
    return {
        "counts": counts,
        "offs": offs[:E],
        "total_cols": total_cols,
        "chunks": chunks,
        "order": order,
    }


def _build_program(cfg, repeat=1):
    import concourse.bass as bass
    import concourse.bacc as bacc
    import concourse.mybir as mybir
    import concourse.tile as tile

    dt = mybir.dt
    COLS = cfg["total_cols"]
    counts = cfg["counts"]
    offs = cfg["offs"]

    nc = bacc.Bacc("TRN2", target_bir_lowering=False, debug=False,
                   num_devices=NCORES)

    # x tokens, p-major: xts[p, do, c] = x_packed[c, do*128+p] (bf16)
    xts = nc.dram_tensor("xts", [128, 8, COLS], dt.bfloat16,
                         kind="ExternalInput").ap()
    # weights pre-permuted on host so each DMA is contiguous per partition:
    # w1s/w3s: (E, p=128, do=8, h=HSLICE); w2s: (E, p=128, ko=HT, d=D)
    w1s = nc.dram_tensor("w1s", [E, 128, 8, HSLICE], dt.bfloat16,
                         kind="ExternalInput").ap()
    w3s = nc.dram_tensor("w3s", [E, 128, 8, HSLICE], dt.bfloat16,
                         kind="ExternalInput").ap()
    w2s = nc.dram_tensor("w2s", [E, 128, HT, D], dt.bfloat16,
                         kind="ExternalInput").ap()
    # partial output, D-major: outd[p, dj, c] = out_partial[c, dj*128+p]
    outd = nc.dram_tensor("outd", [128, 8, COLS], dt.bfloat16,
                          kind="ExternalOutput").ap()

    with tile.TileContext(nc) as tc:
        with (
            tc.tile_pool(name="xpool", bufs=3) as xpool,
            tc.tile_pool(name="wpool", bufs=2) as wpool,
            tc.tile_pool(name="h2pool", bufs=2) as h2pool,
            tc.tile_pool(name="sgpool", bufs=3) as sgpool,
            tc.tile_pool(name="obpool", bufs=3) as obpool,
            tc.tile_pool(name="psgu", bufs=2, space="PSUM") as psgu,
            tc.tile_pool(name="pso", bufs=2, space="PSUM") as pso,
        ):
          for _rep in range(repeat):
            for e in cfg["order"]:
                ce = counts[e]
                xe = xpool.tile([128, 8, 1024], dt.bfloat16, tag="xe")
                nc.sync.dma_start(xe[:, :, :ce], xts[:, :, offs[e]:offs[e] + ce])
                w1t = wpool.tile([128, 8, HSLICE], dt.bfloat16, tag="w1t")
                nc.sync.dma_start(w1t, w1s[e])
                w3t = wpool.tile([128, 8, HSLICE], dt.bfloat16, tag="w3t")
                nc.sync.dma_start(w3t, w3s[e])
                w2t = wpool.tile([128, HT, D], dt.bfloat16, tag="w2t")
                nc.sync.dma_start(w2t, w2s[e])

                for (col0, w) in cfg["chunks"][e]:
                    rel0 = col0 - offs[e]
                    h2 = h2pool.tile([128, HT, 512], dt.bfloat16, tag="h2")
                    for h in range(HT):
                        pg = psgu.tile([128, 512], dt.float32, tag="pg")
                        pu = psgu.tile([128, 512], dt.float32, tag="pu")
                        for d in range(8):
                            nc.tensor.matmul(
                                pg[:, :w],
                                w1t[:, d, h * 128:(h + 1) * 128],
                                xe[:, d, rel0:rel0 + w],
                                start=(d == 0), stop=(d == 7))
                        for d in range(8):
                            nc.tensor.matmul(
                                pu[:, :w],
                                w3t[:, d, h * 128:(h + 1) * 128],
                                xe[:, d, rel0:rel0 + w],
                                start=(d == 0), stop=(d == 7))
                        sg = sgpool.tile([128, 512], dt.float32, tag="sg")
                        nc.scalar.activation(
                            sg[:, :w], pg[:, :w],
                            mybir.ActivationFunctionType.Silu)
                        nc.vector.tensor_mul(
                            out=h2[:, h, :w],
                            in0=sg[:, :w], in1=pu[:, :w])

                    # GEMM2, token-streaming: out[dj] = sum_k w2[k,dj]^T @ h2[k]
                    obt = obpool.tile([128, 8, 512], dt.bfloat16, tag="obt")
                    for pair in range(4):
                        poA = pso.tile([128, 512], dt.float32, tag="poA")
                        poB = pso.tile([128, 512], dt.float32, tag="poB")
                        djA, djB = 2 * pair, 2 * pair + 1
                        for k in range(HT - 1):
                            nc.tensor.matmul(
                                poA[:, :w],
                                w2t[:, k, djA * 128:(djA + 1) * 128],
                                h2[:, k, :w],
                                start=(k == 0), stop=False)
                            nc.tensor.matmul(
                                poB[:, :w],
                                w2t[:, k, djB * 128:(djB + 1) * 128],
                                h2[:, k, :w],
                                start=(k == 0), stop=False)
                        nc.tensor.matmul(
                            poA[:, :w],
                            w2t[:, HT - 1, djA * 128:(djA + 1) * 128],
                            h2[:, HT - 1, :w], start=False, stop=True)
                        nc.tensor.matmul(
                            poB[:, :w],
                            w2t[:, HT - 1, djB * 128:(djB + 1) * 128],
                            h2[:, HT - 1, :w], start=False, stop=True)
                        nc.vector.tensor_copy(out=obt[:, djA, :w],
                                              in_=poA[:, :w])
                        nc.vector.tensor_copy(out=obt[:, djB, :w],
                                              in_=poB[:, :w])
                    # store via SWDGE (Pool queue): never blocks SP loads
                    nc.gpsimd.dma_start(outd[:, :, col0:col0 + w],
                                        obt[:, :, :w])

    nc.compile()
    return nc


def _get_program(cfg, repeat=1):
    key = (tuple(cfg["counts"]), repeat)
    if key not in _COMPILE_CACHE:
        _COMPILE_CACHE[key] = _build_program(cfg, repeat)
    return _COMPILE_CACHE[key]


def _pack_inputs(x, counts, w1, w2, w3, cfg):
    """Build per-core input maps (host-side routing + layout)."""
    offs, COLS = cfg["offs"], cfg["total_cols"]

    # packed x: all valid tokens, exactly packed per expert
    xpack = np.zeros((COLS, D), np.float32)
    starts = np.concatenate([[0], np.cumsum(counts)]).astype(np.int64)
    for e in range(E):
        c = int(counts[e])
        if c:
            xpack[offs[e]:offs[e] + c] = x[starts[e]:starts[e] + c]
    # p-major: xts[p, do, c] = xpack[c, do*128+p]
    xts = np.ascontiguousarray(
        xpack.T.astype(BF16).reshape(8, 128, COLS).transpose(1, 0, 2))

    # weights: transpose so the contraction dim leads, pad H to 3072,
    # slice per core
    w1b = w1.astype(BF16)
    w3b = w3.astype(BF16)
    w2b = w2.astype(BF16)
    # (E, D, Hpad)
    w1T = np.zeros((E, D, NCORES * HSLICE), BF16)
    w1T[:, :, :H] = np.transpose(w1b, (0, 2, 1))
    w3T = np.zeros((E, D, NCORES * HSLICE), BF16)
    w3T[:, :, :H] = np.transpose(w3b, (0, 2, 1))
    # (E, Hpad, D)
    w2T = np.zeros((E, NCORES * HSLICE, D), BF16)
    w2T[:, :H, :] = np.transpose(w2b, (0, 2, 1))

    in_maps = []
    for c in range(NCORES):
        hs = slice(c * HSLICE, (c + 1) * HSLICE)
        # permute so partition p's data is contiguous in DRAM:
        # w1/w3: (D, HSLICE) -> (do=8, p=128, h) -> (p, do, h)
        w1c = w1T[:, :, hs].reshape(E, 8, 128, HSLICE).transpose(0, 2, 1, 3)
        w3c = w3T[:, :, hs].reshape(E, 8, 128, HSLICE).transpose(0, 2, 1, 3)
        # w2: (HSLICE, D) -> (ko=HT, p=128, d) -> (p, ko, d)
        w2c = w2T[:, hs, :].reshape(E, HT, 128, D).transpose(0, 2, 1, 3)
        in_maps.append({
            "xts": xts,
            "w1s": np.ascontiguousarray(w1c),
            "w3s": np.ascontiguousarray(w3c),
            "w2s": np.ascontiguousarray(w2c),
        })
    return in_maps, starts


def _unpack_output(results, counts, cfg, starts):
    offs = cfg["offs"]
    COLS = cfg["total_cols"]
    acc = np.zeros((COLS, D), np.float32)
    for r in results:
        # outd[p, dj, c] -> partial[c, dj*128+p]
        o = r["outd"].astype(np.float32)          # (128, 8, COLS)
        acc += o.transpose(2, 1, 0).reshape(COLS, D)
    out = np.zeros((T, D), np.float32)
    for e in range(E):
        c = int(counts[e])
        if c:
            out[starts[e]:starts[e] + c] = acc[offs[e]:offs[e] + c]
    return out


def kernel(x, num_tokens_per_expert, w1, w2, w3):
    global LAST_RESULTS
    counts = np.asarray(num_tokens_per_expert).astype(np.int64)
    cfg = _derive_cfg(counts)
    if cfg["total_cols"] == 0:
        return np.zeros((T, D), np.float32)

    nc = _get_program(cfg)
    in_maps, starts = _pack_inputs(
        np.asarray(x, np.float32), counts,
        np.asarray(w1, np.float32), np.asarray(w2, np.float32),
        np.asarray(w3, np.float32), cfg)

    from concourse.bass_utils import run_bass_kernel_spmd
    res = run_bass_kernel_spmd(nc, in_maps, list(range(NCORES)))
    LAST_RESULTS = res
    return _unpack_output(res.results, counts, cfg, starts)


# revision 6
# speedup vs baseline: 1.5122x; 1.5122x over previous
"""Grouped-experts SwiGLU MoE kernel for Trainium2 (8 NeuronCores).

Problem: T=8192 tokens (pre-sorted into contiguous per-expert blocks of
sizes num_tokens_per_expert), D=1024, H=2816, E=8 experts.
out[t] = (silu(x@w1^T) * (x@w3^T)) @ w2^T  with the owning expert's weights;
tokens past sum(counts) produce zeros.

Sharding: 8-way tensor-parallel split of the hidden dim H (padded
2816 -> 3072 = 24 tiles of 128; each core owns 3 h-tiles of every expert).
Every core processes ALL valid tokens of ALL experts for its h-slice and
emits partial outputs (contraction over h is split); the host sums the 8
partials.  This makes every core's instruction stream identical (true SPMD)
while doing only token-proportional work per expert -- perfectly
load-balanced regardless of how unbalanced the expert counts are.

Schedule notes (v2):
- GEMM2 is token-streaming (stationary w2 128x128 tiles, streaming h2), so
  its PE cost is 24*tokens cycles with no ceil(count/128) quantization, and
  it is emitted per ~512-token chunk right after that chunk's h2 is ready.
- Output stores go through the SWDGE (gpsimd/Pool) queue so they never
  head-of-line-block the next expert's loads on the SP HWDGE queue.
- Expert order interleaves large and small counts so a large expert's
  compute sits at the rep tail and hides the next rep's prefetch.
- GEMMs run in bf16 (PE 1 cycle/column) with fp32 PSUM accumulation.
"""

import sys

sys.path.insert(0, "/opt/trn_rl_repo")

import numpy as np
import ml_dtypes

T, D, E = 8192, 1024, 8
H = 2816
CAP = T // E
NCORES = 8
HT = 3  # h-tiles of 128 per core (24 total, 22 real + 2 zero pads)
HSLICE = HT * 128  # 384
BF16 = ml_dtypes.bfloat16

_COMPILE_CACHE = {}
LAST_RESULTS = None  # BassKernelResults of the most recent device run


def _derive_cfg(counts):
    """Static structure derived from the per-expert token counts.
    Tokens are packed exactly (no padding): expert e owns packed columns
    [offs[e], offs[e]+counts[e]).  GEMM1/3 and GEMM2 consume near-equal
    chunks of <=512 columns."""
    counts = [int(c) for c in counts]
    offs = [0]
    for c in counts:
        offs.append(offs[-1] + c)
    total_cols = offs[-1]
    chunks = {}   # e -> [(col0, width<=512)]
    for e in range(E):
        c = counts[e]
        chunks[e] = []
        if c == 0:
            continue
        n = -(-c // 512)
        base, rem = divmod(c, n)
        c0 = 0
        for i in range(n):
            w = base + (1 if i < rem else 0)
            chunks[e].append((offs[e] + c0, w))
            c0 += w
    # big, small, big, ... so a large expert sits at the rep tail
    desc = sorted(range(E), key=lambda e: -counts[e])
    order = []
    lo, hi = 0, E - 1
    while lo <= hi:
        order.append(desc[lo])
        if hi != lo:
            order.append(desc[hi])
        lo += 1
        hi -= 1
    order = [e for e in order if counts[e] > 0]
    return {
        "counts": counts,
        "offs": offs[:E],
        "total_cols": total_cols,
        "chunks": chunks,
        "order": order,
    }


def _build_program(cfg, repeat=1):
    import concourse.bass as bass
    import concourse.bacc as bacc
    import concourse.mybir as mybir
    import concourse.tile as tile

    dt = mybir.dt
    COLS = cfg["total_cols"]
    counts = cfg["counts"]
    offs = cfg["offs"]

    nc = bacc.Bacc("TRN2", target_bir_lowering=False, debug=False,
                   num_devices=NCORES)

    # x tokens, p-major: xts[p, do, c] = x_packed[c, do*128+p] (bf16)
    xts = nc.dram_tensor("xts", [128, 8, COLS], dt.bfloat16,
                         kind="ExternalInput").ap()
    # weights pre-permuted on host so each DMA is contiguous per partition:
    # w1s/w3s: (E, p=128, do=8, h=HSLICE); w2s: (E, p=128, ko=HT, d=D)
    w1s = nc.dram_tensor("w1s", [E, 128, 8, HSLICE], dt.bfloat16,
                         kind="ExternalInput").ap()
    w3s = nc.dram_tensor("w3s", [E, 128, 8, HSLICE], dt.bfloat16,
                         kind="ExternalInput").ap()
    w2s = nc.dram_tensor("w2s", [E, 128, HT, D], dt.bfloat16,
                         kind="ExternalInput").ap()
    # partial output: outd[p, c, dj] = out_partial[c, dj*128+p]
    # (token-major per partition so each store is one contiguous 8KB
    # segment per partition instead of 8 strided ~1KB segments)
    outd = nc.dram_tensor("outd", [128, COLS, 8], dt.bfloat16,
                          kind="ExternalOutput").ap()

    with tile.TileContext(nc) as tc:
        with (
            tc.tile_pool(name="xpool", bufs=3) as xpool,
            tc.tile_pool(name="wpool", bufs=2) as wpool,
            tc.tile_pool(name="h2pool", bufs=2) as h2pool,
            tc.tile_pool(name="sgpool", bufs=3) as sgpool,
            tc.tile_pool(name="obpool", bufs=3) as obpool,
            tc.tile_pool(name="psgu", bufs=2, space="PSUM") as psgu,
            tc.tile_pool(name="pso", bufs=2, space="PSUM") as pso,
        ):
          for _rep in range(repeat):
            for e in cfg["order"]:
                ce = counts[e]
                xe = xpool.tile([128, 8, 1024], dt.bfloat16, tag="xe")
                nc.sync.dma_start(xe[:, :, :ce], xts[:, :, offs[e]:offs[e] + ce])
                w1t = wpool.tile([128, 8, HSLICE], dt.bfloat16, tag="w1t")
                nc.sync.dma_start(w1t, w1s[e])
                w3t = wpool.tile([128, 8, HSLICE], dt.bfloat16, tag="w3t")
                nc.sync.dma_start(w3t, w3s[e])
                w2t = wpool.tile([128, HT, D], dt.bfloat16, tag="w2t")
                nc.sync.dma_start(w2t, w2s[e])

                for (col0, w) in cfg["chunks"][e]:
                    rel0 = col0 - offs[e]
                    h2 = h2pool.tile([128, HT, 512], dt.bfloat16, tag="h2")
                    for h in range(HT):
                        pg = psgu.tile([128, 512], dt.float32, tag="pg")
                        pu = psgu.tile([128, 512], dt.float32, tag="pu")
                        for d in range(8):
                            nc.tensor.matmul(
                                pg[:, :w],
                                w1t[:, d, h * 128:(h + 1) * 128],
                                xe[:, d, rel0:rel0 + w],
                                start=(d == 0), stop=(d == 7))
                        for d in range(8):
                            nc.tensor.matmul(
                                pu[:, :w],
                                w3t[:, d, h * 128:(h + 1) * 128],
                                xe[:, d, rel0:rel0 + w],
                                start=(d == 0), stop=(d == 7))
                        sg = sgpool.tile([128, 512], dt.float32, tag="sg")
                        nc.scalar.activation(
                            sg[:, :w], pg[:, :w],
                            mybir.ActivationFunctionType.Silu)
                        nc.vector.tensor_mul(
                            out=h2[:, h, :w],
                            in0=sg[:, :w], in1=pu[:, :w])

                    # GEMM2, token-streaming: out[dj] = sum_k w2[k,dj]^T @ h2[k]
                    obt = obpool.tile([128, 512, 8], dt.bfloat16, tag="obt")
                    for pair in range(4):
                        poA = pso.tile([128, 512], dt.float32, tag="poA")
                        poB = pso.tile([128, 512], dt.float32, tag="poB")
                        djA, djB = 2 * pair, 2 * pair + 1
                        for k in range(HT - 1):
                            nc.tensor.matmul(
                                poA[:, :w],
                                w2t[:, k, djA * 128:(djA + 1) * 128],
                                h2[:, k, :w],
                                start=(k == 0), stop=False)
                            nc.tensor.matmul(
                                poB[:, :w],
                                w2t[:, k, djB * 128:(djB + 1) * 128],
                                h2[:, k, :w],
                                start=(k == 0), stop=False)
                        nc.tensor.matmul(
                            poA[:, :w],
                            w2t[:, HT - 1, djA * 128:(djA + 1) * 128],
                            h2[:, HT - 1, :w], start=False, stop=True)
                        nc.tensor.matmul(
                            poB[:, :w],
                            w2t[:, HT - 1, djB * 128:(djB + 1) * 128],
                            h2[:, HT - 1, :w], start=False, stop=True)
                        nc.vector.tensor_copy(out=obt[:, :w, djA],
                                              in_=poA[:, :w])
                        nc.vector.tensor_copy(out=obt[:, :w, djB],
                                              in_=poB[:, :w])
                    # store via the ACT HWDGE ring: doesn't head-of-line
                    # block the SP ring that carries the weight/x loads
                    nc.scalar.dma_start(outd[:, col0:col0 + w, :],
                                        obt[:, :w, :])

    nc.compile()
    return nc


def _get_program(cfg, repeat=1):
    key = (tuple(cfg["counts"]), repeat)
    if key not in _COMPILE_CACHE:
        _COMPILE_CACHE[key] = _build_program(cfg, repeat)
    return _COMPILE_CACHE[key]


def _pack_inputs(x, counts, w1, w2, w3, cfg):
    """Build per-core input maps (host-side routing + layout)."""
    offs, COLS = cfg["offs"], cfg["total_cols"]

    # packed x: all valid tokens, exactly packed per expert
    xpack = np.zeros((COLS, D), np.float32)
    starts = np.concatenate([[0], np.cumsum(counts)]).astype(np.int64)
    for e in range(E):
        c = int(counts[e])
        if c:
            xpack[offs[e]:offs[e] + c] = x[starts[e]:starts[e] + c]
    # p-major: xts[p, do, c] = xpack[c, do*128+p]
    xts = np.ascontiguousarray(
        xpack.T.astype(BF16).reshape(8, 128, COLS).transpose(1, 0, 2))

    # weights: transpose so the contraction dim leads, pad H to 3072,
    # slice per core
    w1b = w1.astype(BF16)
    w3b = w3.astype(BF16)
    w2b = w2.astype(BF16)
    # (E, D, Hpad)
    w1T = np.zeros((E, D, NCORES * HSLICE), BF16)
    w1T[:, :, :H] = np.transpose(w1b, (0, 2, 1))
    w3T = np.zeros((E, D, NCORES * HSLICE), BF16)
    w3T[:, :, :H] = np.transpose(w3b, (0, 2, 1))
    # (E, Hpad, D)
    w2T = np.zeros((E, NCORES * HSLICE, D), BF16)
    w2T[:, :H, :] = np.transpose(w2b, (0, 2, 1))

    in_maps = []
    for c in range(NCORES):
        hs = slice(c * HSLICE, (c + 1) * HSLICE)
        # permute so partition p's data is contiguous in DRAM:
        # w1/w3: (D, HSLICE) -> (do=8, p=128, h) -> (p, do, h)
        w1c = w1T[:, :, hs].reshape(E, 8, 128, HSLICE).transpose(0, 2, 1, 3)
        w3c = w3T[:, :, hs].reshape(E, 8, 128, HSLICE).transpose(0, 2, 1, 3)
        # w2: (HSLICE, D) -> (ko=HT, p=128, d) -> (p, ko, d)
        w2c = w2T[:, hs, :].reshape(E, HT, 128, D).transpose(0, 2, 1, 3)
        in_maps.append({
            "xts": xts,
            "w1s": np.ascontiguousarray(w1c),
            "w3s": np.ascontiguousarray(w3c),
            "w2s": np.ascontiguousarray(w2c),
        })
    return in_maps, starts


def _unpack_output(results, counts, cfg, starts):
    offs = cfg["offs"]
    COLS = cfg["total_cols"]
    acc = np.zeros((COLS, D), np.float32)
    for r in results:
        # outd[p, c, dj] -> partial[c, dj*128+p]
        o = r["outd"].astype(np.float32)          # (128, COLS, 8)
        acc += o.transpose(1, 2, 0).reshape(COLS, D)
    out = np.zeros((T, D), np.float32)
    for e in range(E):
        c = int(counts[e])
        if c:
            out[starts[e]:starts[e] + c] = acc[offs[e]:offs[e] + c]
    return out


def kernel(x, num_tokens_per_expert, w1, w2, w3):
    global LAST_RESULTS
    counts = np.asarray(num_tokens_per_expert).astype(np.int64)
    cfg = _derive_cfg(counts)
    if cfg["total_cols"] == 0:
        return np.zeros((T, D), np.float32)

    nc = _get_program(cfg)
    in_maps, starts = _pack_inputs(
        np.asarray(x, np.float32), counts,
        np.asarray(w1, np.float32), np.asarray(w2, np.float32),
        np.asarray(w3, np.float32), cfg)

    from concourse.bass_utils import run_bass_kernel_spmd
    res = run_bass_kernel_spmd(nc, in_maps, list(range(NCORES)))
    LAST_RESULTS = res
    return _unpack_output(res.results, counts, cfg, starts)
